# revision 6
# baseline (speedup 1.0000x reference)
"""Trainium2 Bass kernel for nn_CnfProcessingBlock (3-way GAT + type select + relu).

Full (unsharded) inputs in, full output out. Internally:
  - Host prep: fold GAT params (W@a_src etc.), assign nodes to 8 cores
    round-robin per node_type (type-aligned 128 slots), assign each edge to the
    core owning its dst, sort edges by dst slot, and materialize the per-edge
    "halo" stream: [h[src] | 1 | edge_attr] rows grouped into 128-edge chunks
    per 128-dst block (SPMD: identical chunk counts across cores, zero-padded).
  - Device (per core, identical program): for each dst block, build the dst
    one-hots for all chunks with one batched is_equal against an iota tile,
    compute per-edge logits with fused multiply-accumulate ops
    (a_d = onehot x al_d, a_se = stream x [ws|0|we]), leaky-relu + exp,
    scale the stream rows by exp on the scalar engine, and accumulate
    onehot^T @ scaled_rows on the tensor engine (PSUM) -> node-major
    [agg | sum_e]. Normalize by the exp-sum, transpose on PE, apply W_t,
    add residual + bias, relu, DMA out channel-major.
  - Host: unshard [8, 128, M] channel-major slots back to [50000, 128].

Softmax is computed without the per-segment max subtraction: logits here are
O(10) so exp() is safely in fp32 range and the normalization cancels exactly.
"""

import os
import sys
import time

import numpy as np

for _p in ('/opt/trn_rl_repo', '/root/.axon_site/_ro/trn_rl_repo'):
    if os.path.isdir(_p) and _p not in sys.path:
        sys.path.insert(0, _p)

import concourse.bacc as bacc
import concourse.bass as bass
import concourse.mybir as mybir
import concourse.tile as tile
from concourse.bass_utils import run_bass_kernel_spmd

F32 = mybir.dt.float32
ALU = mybir.AluOpType
ACTF = mybir.ActivationFunctionType

P = 128         # partitions / block width / channels
C = 128         # feature channels
ED = 16         # edge-attr dim
CW = 148        # stream row: h(128) | ones(1) | edge_attr(16) | pad(3)
VW = C + 1 + ED  # 145 = logit-vector width
NCORES = 8
TYPES = 3


def _host_prep(h, edge_index, edge_attr, node_type):
    N = h.shape[0]

    nt = np.asarray(node_type).astype(np.int64)
    s_t, offs, idx_t = [], [0], []
    for t in range(TYPES):
        idx = np.nonzero(nt == t)[0]
        idx_t.append(idx)
        st = int(np.ceil(np.ceil(max(len(idx), 1) / NCORES) / P) * P)
        s_t.append(st)
        offs.append(offs[-1] + st)
    M = offs[-1]
    NB = M // P
    type_of_block = np.concatenate(
        [np.full(s_t[t] // P, t, np.int64) for t in range(TYPES)])

    core_of = np.empty(N, np.int64)
    slot_of = np.empty(N, np.int64)
    for t in range(TYPES):
        idx = idx_t[t]
        pos = np.arange(len(idx))
        core_of[idx] = pos % NCORES
        slot_of[idx] = offs[t] + pos // NCORES

    h = np.asarray(h, np.float32)
    edge_attr = np.asarray(edge_attr, np.float32)
    h_cm = np.zeros((NCORES, C, M), np.float32)
    h_cm[core_of, :, slot_of] = h

    src = np.asarray(edge_index[0]).astype(np.int64)
    dst = np.asarray(edge_index[1]).astype(np.int64)
    ecore = core_of[dst]
    dslot = slot_of[dst]
    blk = dslot // P

    cnt = np.zeros((NCORES, NB), np.int64)
    np.add.at(cnt, (ecore, blk), 1)
    K = np.maximum(1, np.ceil(cnt.max(axis=0) / P)).astype(np.int64)
    c0 = np.concatenate([[0], np.cumsum(K)]).astype(np.int64)
    TC = int(c0[-1])

    stream = np.zeros((NCORES, TC * P, CW), np.float32)
    drel = np.full((NCORES, TC * P), -1.0, np.float32)
    for c in range(NCORES):
        m = ecore == c
        es, ed, eb, ea = src[m], dslot[m], blk[m], edge_attr[m]
        order = np.argsort(ed, kind='stable')
        es, ed, eb, ea = es[order], ed[order], eb[order], ea[order]
        starts = np.searchsorted(eb, np.arange(NB))
        rank = np.arange(len(eb)) - starts[eb]
        slot = c0[eb] * P + rank
        stream[c, slot, 0:C] = h[es]
        stream[c, slot, C] = 1.0
        stream[c, slot, C + 1:C + 1 + ED] = ea
        drel[c, slot] = (ed % P).astype(np.float32)

    stream_cm = np.ascontiguousarray(
        stream.reshape(NCORES, TC, P, CW).transpose(0, 2, 1, 3).reshape(NCORES, P, TC * CW))
    drel_cm = np.ascontiguousarray(
        drel.reshape(NCORES, TC, P).transpose(0, 2, 1))

    meta = dict(M=M, NB=NB, TC=TC, K=K, c0=c0, type_of_block=type_of_block,
                core_of=core_of, slot_of=slot_of, offs=offs, N=N)
    return meta, stream_cm, drel_cm, h_cm


def _fold_params(inputs):
    ws, wd, we, Wm, bm = [], [], [], [], []
    for g in ('v', 'r', 'i'):
        W = np.asarray(inputs['W' + g], np.float32)
        ws.append(W @ np.asarray(inputs['as' + g], np.float32))
        wd.append(W @ np.asarray(inputs['ad' + g], np.float32))
        we.append(np.asarray(inputs['We' + g], np.float32)
                  @ np.asarray(inputs['ae' + g], np.float32))
        Wm.append(W)
        bm.append(np.asarray(inputs['b' + g], np.float32))
    vecs = np.zeros((TYPES, CW), np.float32)
    vecs[:, 0:C] = np.stack(ws)
    vecs[:, C + 1:C + 1 + ED] = np.stack(we)
    vecs_rep = np.ascontiguousarray(
        np.broadcast_to(vecs.reshape(1, TYPES, CW), (P, TYPES, CW)).reshape(P, TYPES * CW))
    Wmat = np.ascontiguousarray(
        np.stack(Wm).transpose(1, 0, 2).reshape(C, TYPES * C))
    b3 = np.ascontiguousarray(np.stack(bm).T)
    wd3 = np.ascontiguousarray(np.stack(wd).T)
    return vecs_rep, Wmat, b3, wd3


def _ap3(ap2, inner):
    """Append a dim to a 2-D AP: [[pstep,P],[s,n]] -> [[pstep,P],[s,n],inner]."""
    return bass.AP(ap2.tensor, ap2.offset, list(ap2.ap) + [inner])


def _build_program(meta, batched_iseq=True):
    M, NB, TC = meta['M'], meta['NB'], meta['TC']
    K, c0, tob = meta['K'], meta['c0'], meta['type_of_block']
    offs = meta['offs']
    Kmax = int(K.max())
    OGRP = 4

    nc = bacc.Bacc('TRN2', target_bir_lowering=False, debug=False,
                   num_devices=NCORES)

    d_stream = nc.dram_tensor('stream', [P, TC * CW], F32, kind='ExternalInput')
    d_drel = nc.dram_tensor('drel', [P, TC], F32, kind='ExternalInput')
    d_hcm = nc.dram_tensor('h_cm', [P, M], F32, kind='ExternalInput')
    d_vecs = nc.dram_tensor('vecs_rep', [P, TYPES * CW], F32, kind='ExternalInput')
    d_wmat = nc.dram_tensor('Wmat', [P, TYPES * C], F32, kind='ExternalInput')
    d_b3 = nc.dram_tensor('b3', [P, TYPES], F32, kind='ExternalInput')
    d_wd3 = nc.dram_tensor('wd3', [P, TYPES], F32, kind='ExternalInput')
    d_out = nc.dram_tensor('out', [P, M], F32, kind='ExternalOutput')

    with tile.TileContext(nc) as tc:
        with (
            tc.tile_pool(name='const', bufs=1) as constp,
            tc.tile_pool(name='stream', bufs=3) as streamp,
            tc.tile_pool(name='oh', bufs=3) as ohp,
            tc.tile_pool(name='work', bufs=4) as workp,
            tc.tile_pool(name='junk', bufs=4) as junkp,
            tc.tile_pool(name='tail', bufs=3) as tailp,
            tc.tile_pool(name='pfeat', bufs=3, space='PSUM') as pfeat,
            tc.tile_pool(name='pbig', bufs=3, space='PSUM') as pbig,
            tc.tile_pool(name='prowp', bufs=1, space='PSUM') as prowp,
        ):
            h_sb = constp.tile([P, M], F32)
            nc.sync.dma_start(out=h_sb[:], in_=d_hcm[:])
            drel_sb = constp.tile([P, TC], F32)
            nc.sync.dma_start(out=drel_sb[:], in_=d_drel[:])
            vecs_sb = constp.tile([P, TYPES * CW], F32)
            nc.sync.dma_start(out=vecs_sb[:], in_=d_vecs[:])
            wm_sb = constp.tile([P, TYPES * C], F32)
            nc.sync.dma_start(out=wm_sb[:], in_=d_wmat[:])
            b3_sb = constp.tile([P, TYPES], F32)
            nc.sync.dma_start(out=b3_sb[:], in_=d_b3[:])
            wd3_sb = constp.tile([P, TYPES], F32)
            nc.sync.dma_start(out=wd3_sb[:], in_=d_wd3[:])

            # iota row (0..127 along free) and identity built without
            # affine_select / custom-DVE (neither survives this runtime).
            iota_i = constp.tile([P, P], mybir.dt.int32)
            nc.gpsimd.iota(iota_i[:], pattern=[[1, P]], base=0,
                           channel_multiplier=0)
            iota_f = constp.tile([P, P], F32)
            nc.vector.tensor_copy(out=iota_f[:], in_=iota_i[:])
            iotac_i = constp.tile([P, P], mybir.dt.int32)
            nc.gpsimd.iota(iotac_i[:], pattern=[[0, P]], base=0,
                           channel_multiplier=1)
            iotac_f = constp.tile([P, P], F32)
            nc.vector.tensor_copy(out=iotac_f[:], in_=iotac_i[:])
            ident = constp.tile([P, P], F32)
            nc.vector.tensor_tensor(out=ident[:], in0=iotac_f[:],
                                    in1=iota_f[:], op=ALU.is_equal)
            ones_row = constp.tile([1, P], F32)
            nc.vector.tensor_scalar(out=ones_row[:], in0=iota_f[0:1, :],
                                    scalar1=0.0, scalar2=1.0,
                                    op0=ALU.mult, op1=ALU.add)

            # al_d replicated across partitions: al_d[d] = h[:, d] . wd_{type(d)}
            ald_sb = constp.tile([P, M], F32)
            for t in range(TYPES):
                for j0 in range(offs[t], offs[t + 1], 512):
                    w = min(512, offs[t + 1] - j0)
                    prow = prowp.tile([1, 512], F32, tag='prow')
                    nc.tensor.matmul(prow[:1, :w], lhsT=wd3_sb[:, t:t + 1],
                                     rhs=h_sb[:, j0:j0 + w], start=True, stop=True)
                    row = workp.tile([1, 512], F32, tag='aldrow')
                    nc.vector.tensor_copy(out=row[:1, :w], in_=prow[:1, :w])
                    prep = pbig.tile([P, 512], F32, tag='big')
                    nc.tensor.matmul(prep[:, :w], lhsT=ones_row[:],
                                     rhs=row[:1, :w], start=True, stop=True)
                    nc.vector.tensor_copy(out=ald_sb[:, j0:j0 + w],
                                          in_=prep[:, :w])

            for b in range(NB):
                t = int(tob[b])
                Kb = int(K[b])
                cb = int(c0[b])
                blkt = streamp.tile([P, Kmax * CW], F32, tag='stream')
                nc.sync.dma_start(
                    out=blkt[:, :Kb * CW],
                    in_=d_stream[:, cb * CW:(cb + Kb) * CW])

                # one-hot for every chunk of the block: [P, Kb*P],
                # oh[:, k*P+d] = (drel[:, cb+k] == d)
                oh_all = ohp.tile([P, Kmax * P], F32, tag='oh')
                if batched_iseq:
                    in0 = _ap3(drel_sb[:, cb:cb + Kb], [0, P])
                    it = iota_f[:]
                    in1 = bass.AP(it.tensor, it.offset,
                                  [list(it.ap[0]), [0, Kb], list(it.ap[1])])
                    o = oh_all[:, 0:Kb * P]
                    outap = bass.AP(o.tensor, o.offset,
                                    [list(o.ap[0]), [P, Kb], [1, P]])
                    nc.vector.tensor_tensor(out=outap, in0=in0, in1=in1,
                                            op=ALU.is_equal)
                else:
                    for k in range(Kb):
                        nc.vector.tensor_tensor(
                            out=oh_all[:, k * P:(k + 1) * P],
                            in0=drel_sb[:, cb + k:cb + k + 1].to_broadcast([P, P]),
                            in1=iota_f[:], op=ALU.is_equal)

                ad_blk = workp.tile([P, Kmax], F32, tag='ad')
                ase_blk = workp.tile([P, Kmax], F32, tag='ase')
                for k in range(Kb):
                    j1 = junkp.tile([P, P], F32, tag='j1')
                    nc.vector.scalar_tensor_tensor(
                        out=j1[:], in0=oh_all[:, k * P:(k + 1) * P],
                        scalar=1.0, in1=ald_sb[:, b * P:(b + 1) * P],
                        op0=ALU.mult, op1=ALU.mult,
                        accum_out=ad_blk[:, k:k + 1])
                    j2 = junkp.tile([P, VW], F32, tag='j2')
                    nc.vector.scalar_tensor_tensor(
                        out=j2[:], in0=blkt[:, k * CW:k * CW + VW],
                        scalar=1.0, in1=vecs_sb[:, t * CW:t * CW + VW],
                        op0=ALU.mult, op1=ALU.mult,
                        accum_out=ase_blk[:, k:k + 1])

                a_blk = workp.tile([P, Kmax], F32, tag='a')
                nc.vector.tensor_tensor(out=a_blk[:, :Kb], in0=ad_blk[:, :Kb],
                                        in1=ase_blk[:, :Kb], op=ALU.add)
                e_in = workp.tile([P, Kmax], F32, tag='ein')
                nc.vector.scalar_tensor_tensor(
                    out=e_in[:, :Kb], in0=a_blk[:, :Kb], scalar=0.2,
                    in1=a_blk[:, :Kb], op0=ALU.mult, op1=ALU.max)
                e_blk = workp.tile([P, Kmax], F32, tag='e')
                nc.scalar.activation(out=e_blk[:, :Kb], in_=e_in[:, :Kb],
                                     func=ACTF.Exp)

                feat = pfeat.tile([P, C + 1], F32, tag='feat')
                for k in range(Kb):
                    sts = workp.tile([P, C + 1], F32, tag='sts')
                    nc.scalar.activation(
                        out=sts[:], in_=blkt[:, k * CW:k * CW + C + 1],
                        func=ACTF.Copy, scale=e_blk[:, k:k + 1])
                    nc.tensor.matmul(feat[:], lhsT=oh_all[:, k * P:(k + 1) * P],
                                     rhs=sts[:],
                                     start=(k == 0), stop=(k == Kb - 1))

                sden = tailp.tile([P, 1], F32, tag='sden')
                nc.vector.tensor_scalar_add(sden[:], feat[:, C:C + 1], 1e-16)
                rcol = tailp.tile([P, 1], F32, tag='rcol')
                nc.vector.reciprocal(rcol[:], sden[:])
                aggn = tailp.tile([P, P], F32, tag='aggn')
                nc.scalar.activation(out=aggn[:], in_=feat[:, 0:C],
                                     func=ACTF.Copy, scale=rcol[:])
                ptr = pbig.tile([P, 512], F32, tag='big')
                nc.tensor.transpose(ptr[:, :P], aggn[:], ident[:])
                aggcm = tailp.tile([P, P], F32, tag='aggcm')
                nc.scalar.activation(out=aggcm[:], in_=ptr[:, :P],
                                     func=ACTF.Copy)
                pout = pbig.tile([P, 512], F32, tag='big')
                nc.tensor.matmul(pout[:, :P], lhsT=wm_sb[:, t * C:(t + 1) * C],
                                 rhs=aggcm[:], start=True, stop=True)
                res = tailp.tile([P, P], F32, tag='res')
                nc.vector.tensor_tensor(out=res[:], in0=pout[:, :P],
                                        in1=h_sb[:, b * P:(b + 1) * P],
                                        op=ALU.add)
                ob = b % OGRP
                if ob == 0:
                    outw = tailp.tile([P, OGRP * P], F32, tag='outw')
                nc.scalar.activation(out=outw[:, ob * P:(ob + 1) * P], in_=res[:],
                                     func=ACTF.Relu,
                                     bias=b3_sb[:, t:t + 1], scale=1.0)
                if ob == OGRP - 1 or b == NB - 1:
                    g0 = b - ob
                    nc.sync.dma_start(out=d_out[:, g0 * P:(b + 1) * P],
                                      in_=outw[:, :(ob + 1) * P])

    nc.compile()
    return nc


def kernel(**inputs):
    t0 = time.time()
    meta, stream_cm, drel_cm, h_cm = _host_prep(
        inputs['h'], inputs['edge_index'], inputs['edge_attr'],
        inputs['node_type'])
    vecs_rep, Wmat, b3, wd3 = _fold_params(inputs)
    t1 = time.time()

    nc = _build_program(meta)
    t2 = time.time()

    in_maps = []
    for c in range(NCORES):
        in_maps.append({
            'stream': stream_cm[c], 'drel': drel_cm[c], 'h_cm': h_cm[c],
            'vecs_rep': vecs_rep, 'Wmat': Wmat, 'b3': b3, 'wd3': wd3,
        })
    res = run_bass_kernel_spmd(nc, in_maps, core_ids=list(range(NCORES)))
    kernel.last_results = res
    t3 = time.time()

    core_of, slot_of, N = meta['core_of'], meta['slot_of'], meta['N']
    full = np.empty((N, C), np.float32)
    for c in range(NCORES):
        m = core_of == c
        full[m] = res.results[c]['out'][:, slot_of[m]].T
    if os.environ.get('KERNEL_VERBOSE'):
        print(f'[kernel] prep {t1 - t0:.2f}s build+compile {t2 - t1:.2f}s '
              f'run {t3 - t2:.2f}s', file=sys.stderr)
    return full


kernel.last_results = None
